# revision 33
# baseline (speedup 1.0000x reference)
"""Self-contained TRN2 kernel for the bidirectional attention correction.

kernel(hl, hr) -> (mu_lr, mu_rl), matching:
    hl_n = rownorm(hl); hr_n = rownorm(hr)
    a = hl_n @ hr_n.T
    mu_lr = hr_n - softmax(a, 1).T @ hl_n
    mu_rl = hl_n - softmax(a, 0) @ hr_n

SPMD on 8 NeuronCores: core c owns rows [c*1024,(c+1)*1024) of hl and hr.

Structure (all matmuls fp8 DoubleRow, 157 TF/s):
  prep: bulk-load hl/hr, row-normalize, transpose; AllGather hr_n.T (fp8,
        2 halves, first fired early).
  P1:   per 512-col chunk: a-matmuls, exp into a small rotating buffer,
        column-sum matmuls, slab DMA to the AllToAll staging, and PE
        transposes into the resident exp_aT [j-part, i].  Natural-layout
        exp lives only per-chunk.  The h0-half collectives (ReduceScatter
        of col sums, AllToAll) fire mid-P1.
  P2b:  mu_rl = hl_n - (exp_aT contract @ hrn8s)/S2P -- pure DR matmuls
        (lhsT = exp_aT slices), rhs = 1/s-scaled hr_n fp8 gathered; h0
        contraction first while the h1 collectives land.
  P2a:  mu_lr = hr_n - (a2a_exp contract @ hlp8)/S1 -- lhsT streamed from
        the AllToAll output, rhs = r-scaled hl_n fp8 gathered.
"""

import sys

for _p in ("/opt/trn_rl_repo",):
    if _p not in sys.path:
        sys.path.insert(0, _p)

from contextlib import ExitStack

import numpy as np

import concourse.bass as bass
import concourse.tile as tile
from concourse import bacc, mybir
from concourse.masks import make_identity

F32 = mybir.dt.float32
BF16 = mybir.dt.bfloat16
FP8 = mybir.dt.float8e4

ADD = mybir.AluOpType.add
SUB = mybir.AluOpType.subtract
MULT = mybir.AluOpType.mult
BYPASS = mybir.AluOpType.bypass
EXP = mybir.ActivationFunctionType.Exp
COPY = mybir.ActivationFunctionType.Copy
SQUARE = mybir.ActivationFunctionType.Square
AXL_X = mybir.AxisListType.X
DROW = mybir.MatmulPerfMode.DoubleRow


def build(C=8, NL=1024, M=8192, D=1024, stop_after="full"):
    PB = NL // 128          # local row blocks (i)
    DK = D // 128           # contraction chunks over D
    JB = M // 128           # j 128-blocks
    BLK = M // C            # j-cols per core block (== NL here)
    NLH = NL // 2           # j-cols per half
    W1 = 512                # P1 j-chunk width
    NQ = NLH // W1          # chunks per (half, block) piece
    JC = M // W1            # P1 j-chunks
    DW = 512                # d-chunk width for P2 outputs
    DH = D // DW            # d-halves
    S1 = float(8 * M)       # hl' fp8 scale (P2a rhs)
    S2P = float(8 * M)      # hrn8s fp8 scale (P2b rhs, folded 1/s_j)
    SL = 16.0               # hl_n.T fp8 scale (P1 lhsT)
    SR = 16.0               # hr_n.T fp8 scale (P1 rhs)
    KT = C * PB             # 128-blocks along the full i axis
    PBH = PB // 2
    TR_INLINE = 12          # chunks whose transposes run inside P1
    groups = [list(range(C))]
    LVL = {"prep": 0, "p1": 1, "p2b": 2, "full": 3}[stop_after]
    assert PB % 2 == 0 and NQ == 1

    nc = bacc.Bacc("TRN2", target_bir_lowering=False, debug=False, num_devices=C)

    hl_in = nc.dram_tensor("hl", [NL, D], F32, kind="ExternalInput").ap()
    hr_in = nc.dram_tensor("hr", [NL, D], F32, kind="ExternalInput").ap()
    mu_lr_o = nc.dram_tensor("mu_lr", [NL, D], F32, kind="ExternalOutput").ap()
    mu_rl_o = nc.dram_tensor("mu_rl", [NL, D], F32, kind="ExternalOutput").ap()

    with tile.TileContext(nc) as tc, ExitStack() as ctx:
        dram = ctx.enter_context(tc.tile_pool(name="dram", bufs=1, space="DRAM"))
        sb = ctx.enter_context(tc.tile_pool(name="sb", bufs=1))
        ps = ctx.enter_context(tc.tile_pool(name="ps", bufs=1, space="PSUM"))

        # ---- internal DRAM ----
        hrnT_loc = [dram.tile([D, NLH], FP8, name=f"hrnT_loc{h}")
                    for h in range(2)]
        hrnT_all = [dram.tile([C, D, NLH], FP8, name=f"hrnT_all{h}",
                              addr_space="Shared") for h in range(2)]
        a2a_in = [dram.tile([C, 128, PB, NLH], FP8, name=f"a2a_in{h}")
                  for h in range(2)]
        a2a_out = [dram.tile([C, 128, PB, NLH], FP8, name=f"a2a_out{h}")
                   for h in range(2)]
        s_loc = [dram.tile([C * NLH], F32, name=f"s_loc{h}") for h in range(2)]
        s_red = [dram.tile([NLH], F32, name=f"s_red{h}") for h in range(2)]
        hrn8s_loc = dram.tile([NL, D], FP8)
        hrn8s_all = [dram.tile([C, NLH, D], FP8, name=f"hrn8s_all{h}",
                               addr_space="Shared") for h in range(2)]
        hlp8_loc = dram.tile([NL, D], FP8)
        hlp8_all = dram.tile([C, NL, D], FP8, addr_space="Shared")
        warm = dram.tile([128], F32, name="warm")
        warm_o = dram.tile([C * 128], F32, name="warm_o",
                           addr_space="Shared")

        # ---- SBUF resident ----
        exp_aT = sb.tile([128, JB, NL], FP8, name="exp_aT")    # exp(a).T
        exp_rot = sb.tile([128, PB, 2, W1], FP8, name="exp_rot")
        hl_nb = sb.tile([128, PB, D], BF16, name="hl_nb")      # hl_n
        hrn_b = sb.tile([128, PB, D], BF16, name="hrn_b")      # hr_n local
        hl_nT = sb.tile([128, DK, NL], FP8, name="hl_nT")      # hl_n.T * SL
        bulk = sb.tile([128, PB // 2, D], F32, name="bulk")    # prep load
        rhsT = sb.tile([128, 2, DK, W1], FP8, name="rhsT")     # P1 rhs stream
        rhs_res = sb.tile([128, JB, DW], FP8, name="rhs_res",
                          tag="rhsT")                          # P2b rhs
        hlp8_res = sb.tile([128, KT, D], FP8, name="hlp8_res",
                           tag="exp_aT")                       # P2a rhs
        la_st = sb.tile([128, 2, KT, 128], FP8, name="la_st")  # P2a lhsT
        trT_st = sb.tile([128, 2, DK, 128], FP8, name="trT_st")
        cast_st = sb.tile([128, 4, D], FP8, name="cast_st")
        out_st = sb.tile([128, 3, DW], F32, name="out_st")
        s_row = sb.tile([1, 1, W1], F32, name="s_row")
        # consts / stats
        ident_b = sb.tile([128, 128], BF16, name="ident_b")
        ident_e = sb.tile([128, 128], FP8, name="ident_e")
        ones_e = sb.tile([128, 2, 16], FP8, name="ones_e")
        stats = sb.tile([128, 256], F32, name="stats")
        r_parts = stats[:, 0:PB * JC].rearrange("p (a b) -> p a b", a=PB)
        r_red = stats[:, 128:128 + PB]
        r_red3 = stats[:, 128:128 + PB].rearrange("p (a b) -> p a b", b=1)
        rinv = stats[:, 136:136 + PB]
        s_sb = stats[:, 144:144 + PB]
        srec = stats[:, 152:152 + PB]
        sinv = stats[:, 160:160 + PB]
        nrmp = stats[:, 168:168 + 2 * PB].rearrange(
            "p (a b) -> p a b", a=PB)                          # [128,PB,2]
        nrm1 = stats[:, 184:184 + PB]
        rnv_r = stats[:, 192:192 + PB]                         # 1/|hr row|
        rnv_l = stats[:, 200:200 + PB]                         # 1/|hl row|
        nrm2 = stats[:, 208:208 + PB]
        wsb = stats[:, 216:217]

        make_identity(nc, ident_b)
        nc.vector.tensor_copy(out=ident_e, in_=ident_b)
        nc.vector.memset(ones_e, 1.0)

        # tiny warm-up collective: absorbs the CC-path init cost while the
        # prep pipeline runs
        nc.sync.dma_start(
            out=warm.rearrange("(p a) -> p a", p=128), in_=wsb)
        nc.gpsimd.collective_compute(
            "AllGather", BYPASS, replica_groups=groups,
            ins=[warm.opt()], outs=[warm_o.opt()])

        # ================= prep: bulk norm + transpose ======================
        DQ = [nc.sync, nc.scalar]

        def norm_q(src, half, q, dstb, rnv):
            """Load one 128-row block on 2 queues, normalize (bf16)."""
            pb = half * PBH + q
            for dd in range(DH):
                DQ[dd].dma_start(
                    out=bulk[:, q, dd * DW:(dd + 1) * DW],
                    in_=src[pb * 128:(pb + 1) * 128, dd * DW:(dd + 1) * DW])
                sq = ps.tile([128, DW], F32, tag="acc", bufs=4,
                             name=f"sq{half}_{q}_{dd}")
                nc.scalar.activation(
                    out=sq, in_=bulk[:, q, dd * DW:(dd + 1) * DW],
                    func=SQUARE, accum_out=nrmp[:, pb, dd:dd + 1])
            # per-block latency chain: add, sqrt, recip on [128,1]
            nc.vector.tensor_add(
                out=nrm1[:, pb:pb + 1], in0=nrmp[:, pb, 0:1],
                in1=nrmp[:, pb, 1:2])
            nc.scalar.sqrt(out=nrm2[:, pb:pb + 1], in_=nrm1[:, pb:pb + 1])
            nc.vector.reciprocal(out=rnv[:, pb:pb + 1], in_=nrm2[:, pb:pb + 1])
            nc.vector.tensor_scalar_mul(
                out=dstb[:, pb, :], in0=bulk[:, q, :],
                scalar1=rnv[:, pb:pb + 1])

        def hr_transpose(pb):
            tstage = trT_st[:, pb % 2, :, :]
            for dk in range(DK):
                pst = ps.tile([128, 128], BF16, tag="rot", bufs=3,
                              name=f"ptB{pb}_{dk}")
                nc.tensor.transpose(
                    pst, hrn_b[:, pb, dk * 128:(dk + 1) * 128], ident_b)
                if dk % 2 == 0:
                    nc.scalar.mul(out=tstage[:, dk, :], in_=pst, mul=SR)
                else:
                    nc.vector.tensor_scalar_mul(
                        out=tstage[:, dk, :], in0=pst, scalar1=SR)
            h, pq = divmod(pb, PBH)
            nc.sync.dma_start(
                out=hrnT_loc[h].rearrange("(dk p) j -> p dk j", p=128)
                [:, :, pq * 128:(pq + 1) * 128],
                in_=tstage)

        def hl_transpose(pb):
            for dk in range(DK):
                pst = ps.tile([128, 128], BF16, tag="rot", bufs=3,
                              name=f"ptA{pb}_{dk}")
                nc.tensor.transpose(
                    pst, hl_nb[:, pb, dk * 128:(dk + 1) * 128], ident_b)
                nc.vector.tensor_scalar_mul(
                    out=hl_nT[:, dk, pb * 128:(pb + 1) * 128], in0=pst,
                    scalar1=SL)

        # hr half0 -> gather0 early; then hl half0, hr half1 -> gather1, hl h1
        for q in range(PBH):
            norm_q(hr_in, 0, q, hrn_b, rnv_r)
            hr_transpose(q)
        nc.gpsimd.collective_compute(
            "AllGather", BYPASS, replica_groups=groups,
            ins=[hrnT_loc[0].opt()], outs=[hrnT_all[0].opt()])
        for q in range(PBH):
            norm_q(hl_in, 0, q, hl_nb, rnv_l)
            hl_transpose(q)
        for q in range(PBH):
            norm_q(hr_in, 1, q, hrn_b, rnv_r)
            hr_transpose(PBH + q)
        nc.gpsimd.collective_compute(
            "AllGather", BYPASS, replica_groups=groups,
            ins=[hrnT_loc[1].opt()], outs=[hrnT_all[1].opt()])
        for q in range(PBH):
            norm_q(hl_in, 1, q, hl_nb, rnv_l)
            hl_transpose(PBH + q)

        # ---- post-half-h helpers ------------------------------------------
        def s_rs(h):
            """ReduceScatter the h-half col sums (dispatch only)."""
            nc.gpsimd.collective_compute(
                "ReduceScatter", ADD, replica_groups=groups,
                ins=[s_loc[h].opt()], outs=[s_red[h].opt()])

        def sinv_chain(h, readback=None):
            """s_red[h] -> sinv columns (readback + DVE recip/mul)."""
            pbs = slice(h * PBH, (h + 1) * PBH)
            (readback or nc.scalar).dma_start(
                out=s_sb[:, pbs],
                in_=s_red[h].rearrange("(b p) -> p b", p=128))
            nc.vector.reciprocal(out=srec[:, pbs], in_=s_sb[:, pbs])
            nc.vector.tensor_scalar_mul(
                out=sinv[:, pbs], in0=srec[:, pbs], scalar1=S2P)

        def hrn8s_half(h):
            """Scale own hr_n rows by sinv -> fp8 -> AllGather."""
            hrn8s_rows = hrn8s_loc.rearrange("(pb p) d -> p pb d", p=128)
            for pq in range(PBH):
                pb = h * PBH + pq
                st8 = cast_st[:, 2 + pb % 2, :]
                nc.scalar.activation(
                    out=st8, in_=hrn_b[:, pb, :], func=COPY,
                    scale=sinv[:, pb:pb + 1])
                nc.gpsimd.dma_start(out=hrn8s_rows[:, pb, :], in_=st8)
            nc.gpsimd.collective_compute(
                "AllGather", BYPASS, replica_groups=groups,
                ins=[hrn8s_loc[h * NLH:(h + 1) * NLH, :].opt()],
                outs=[hrn8s_all[h].opt()])

        def load_rhs_half(dh, engine):
            for b in range(C):
                for h in range(2):
                    j0b = b * PB + h * PBH
                    engine.dma_start(
                        out=rhs_res[:, j0b:j0b + PBH, :],
                        in_=hrn8s_all[h][b].rearrange(
                            "(jb p) d -> p jb d", p=128)
                        [:, :, dh * DW:(dh + 1) * DW])

        def a2a_half(h):
            nc.gpsimd.collective_compute(
                "AllToAll", BYPASS, replica_groups=groups,
                ins=[a2a_in[h].opt()], outs=[a2a_out[h].opt()])

        def transpose_chunk(ci):
            """Transpose chunk ci's exp block into exp_aT (PE + DVE/Act)."""
            h, b = divmod(ci, C)
            sl = ci % 2
            for ib in range(PB):
                for jq in range(W1 // 128):
                    jb = (b * BLK + h * NLH) // 128 + jq
                    pst = ps.tile([128, 128, 2], FP8, tag="rot", bufs=3,
                                  name=f"ptb{ci}_{ib}_{jq}")
                    nc.tensor.transpose(
                        pst[:, :, 0],
                        exp_rot[:, ib, sl, jq * 128:(jq + 1) * 128], ident_e)
                    if (ib + jq) % 4 < 3:
                        nc.vector.tensor_copy(
                            out=exp_aT[:, jb, ib * 128:(ib + 1) * 128],
                            in_=pst[:, :, 0])
                    else:
                        nc.scalar.copy(
                            out=exp_aT[:, jb, ib * 128:(ib + 1) * 128],
                            in_=pst[:, :, 0])

        # ================= P1: a-matmul + exp + col sums + transposes =======
        chunks = [(h, b, q) for h in range(2) for b in range(C)
                  for q in range(NQ)]
        if LVL >= 1:
            deferred = []

            def flush_deferred():
                while deferred:
                    deferred.pop(0)()

            psum_s = [None]

            for ci, (h, b, q) in enumerate(chunks):
                j0 = b * BLK + h * NLH + q * W1   # global j of this chunk
                sl = ci % 2
                rt = rhsT[:, sl, :, :]
                nc.sync.dma_start(
                    out=rt,
                    in_=hrnT_all[h][b].rearrange("(dk p) j -> p dk j", p=128)
                    [:, :, q * W1:(q + 1) * W1])
                for ib in range(PB):
                    t_ = ci * PB + ib
                    pa = ps.tile([128, W1], F32,
                                 tag=("rot" if t_ % 7 < 3 else "acc"),
                                 bufs=(3 if t_ % 7 < 3 else 4),
                                 name=f"pa{ci}_{ib}")
                    for dkp in range(DK // 2):
                        nc.tensor.matmul(
                            pa,
                            lhsT=hl_nT[:, 2 * dkp:2 * dkp + 2,
                                       ib * 128:(ib + 1) * 128],
                            rhs=rt[:, 2 * dkp:2 * dkp + 2, :],
                            start=(dkp == 0), stop=(dkp == DK // 2 - 1),
                            perf_mode=DROW)
                    et = exp_rot[:, ib, sl, :]
                    nc.scalar.activation(
                        out=et, in_=pa, func=EXP, scale=1.0 / (SL * SR),
                        accum_out=r_parts[:, ib, ci:ci + 1])
                    if ib % 2 == 0:
                        flush_deferred()
                        continue

                    def ones_mm(ci=ci, ib=ib, h=h, b=b, sl=sl):
                        ibp = ib // 2
                        if ibp == 0:
                            psum_s[0] = ps.tile([1, W1], F32, tag="colsum",
                                                bufs=1, name=f"pscs{ci}")
                        # DoubleRow column-sum over an i-pair
                        nc.tensor.matmul(
                            psum_s[0], lhsT=ones_e[:, :, 0:1],
                            rhs=exp_rot[:, ib - 1:ib + 1, sl, :],
                            start=(ibp == 0), stop=(ibp == PB // 2 - 1),
                            perf_mode=DROW)
                        if ib == PB - 1:
                            sr = s_row[:, 0, :]
                            nc.vector.tensor_copy(out=sr, in_=psum_s[0])
                            nc.sync.dma_start(
                                out=s_loc[h][b * W1:(b + 1) * W1].rearrange(
                                    "(a b) -> a b", a=1),
                                in_=sr)

                    flush_deferred()
                    deferred.append(ones_mm)
                # exp chunk -> AllToAll staging slab
                nc.scalar.dma_start(
                    out=a2a_in[h][b],
                    in_=exp_rot[:, :, sl, :])
                flush_deferred()
                if ci < TR_INLINE:
                    transpose_chunk(ci)
                if ci == C - 1:
                    # h0 fully done: dispatch-safe h0 collectives during P1
                    s_rs(0)
                    a2a_half(0)

            # --- P1 end: per-engine-queue ordering matters here. ---
            s_rs(1)                      # gpsimd; CC after A2A-h0
            sinv_chain(0)                # scalar readback + DVE (deps met)
            hrn8s_half(0)                # Act casts h0 + AG-h0 dispatch
            load_rhs_half(0, engine=nc.sync)   # fires when AG-h0 lands
            for ci in range(TR_INLINE, JC):
                transpose_chunk(ci)      # fills the CC gap on PE
            # h1 chain after the transpose copies so the RS1 wait does not
            # head-of-line-block the Act/DVE queues
            sinv_chain(1, readback=nc.sync)
            hrn8s_half(1)                # Act casts h1 + AG-h1 dispatch

            # r -> rinv ; hl' fp8 -> DRAM -> gather (P2a rhs). DVE + sync.
            nc.vector.tensor_reduce(out=r_red3, in_=r_parts, op=ADD, axis=AXL_X)
            nc.vector.reciprocal(out=rinv, in_=r_red)
            hlp8_rows = hlp8_loc.rearrange("(pb p) d -> p pb d", p=128)
            for ib in range(PB):
                st8 = cast_st[:, ib % 2, :]
                nc.vector.tensor_scalar(
                    out=st8, in0=hl_nb[:, ib, :],
                    scalar1=rinv[:, ib:ib + 1], scalar2=S1, op0=MULT, op1=MULT)
                nc.sync.dma_start(out=hlp8_rows[:, ib, :], in_=st8)
            nc.gpsimd.collective_compute(
                "AllGather", BYPASS, replica_groups=groups,
                ins=[hlp8_loc.opt()], outs=[hlp8_all.opt()])
            a2a_half(1)

        # ====== P2b: mu_rl = hl_n - (exp_aT contract @ hrn8s)/S2P ===========
        # jbp order: h0 rows first so the h1 collectives can land meanwhile
        jbp_order = ([jbp for jbp in range(JB // 2) if jbp % PBH < 2]
                     + [jbp for jbp in range(JB // 2) if jbp % PBH >= 2])

        def load_la(jb):
            la = la_st[:, jb % 2, :, :]
            h, jo = divmod(jb, PBH)
            for src in range(C):
                nc.scalar.dma_start(
                    out=la[:, src * PB:(src + 1) * PB, :],
                    in_=a2a_out[h][src][:, :, jo * 128:(jo + 1) * 128])

        if LVL >= 2:
            for dh in range(DH):
                if dh > 0:
                    load_rhs_half(dh, engine=nc.sync)
                    if LVL >= 3:
                        # prefetch P2a streams while dh=1 computes: rhs
                        # (into the freeing exp_aT slot) + first 2 lhsT blocks
                        for src in range(C):
                            DQ[src % 2].dma_start(
                                out=hlp8_res[:, src * PB:(src + 1) * PB, :],
                                in_=hlp8_all[src].rearrange(
                                    "(ib p) d -> p ib d", p=128))
                        load_la(0)
                        load_la(1)
                for ib in range(PB):
                    acc = ps.tile([128, DW], F32, tag="acc", bufs=4,
                                  name=f"acc{dh}_{ib}")
                    for jx, jbp in enumerate(jbp_order):
                        # DoubleRow: contraction over a jb-pair (K=256)
                        nc.tensor.matmul(
                            acc,
                            lhsT=exp_aT[:, 2 * jbp:2 * jbp + 2,
                                        ib * 128:(ib + 1) * 128],
                            rhs=rhs_res[:, 2 * jbp:2 * jbp + 2, :],
                            start=(jx == 0), stop=(jx == JB // 2 - 1),
                            perf_mode=DROW)
                    st = out_st[:, (dh * PB + ib) % 3, :DW]
                    nc.scalar.activation(
                        out=st, in_=acc, func=COPY, scale=-1.0 / S2P)
                    nc.vector.tensor_add(
                        out=st, in0=st,
                        in1=hl_nb[:, ib, dh * DW:(dh + 1) * DW])
                    nc.gpsimd.dma_start(
                        out=mu_rl_o[ib * 128:(ib + 1) * 128,
                                    dh * DW:(dh + 1) * DW], in_=st)

        # ====== P2a: mu_lr = hr_n - (exp contract @ hlp8)/S1 ================
        if LVL >= 3:
            for jb in range(PB):
                la = la_st[:, jb % 2, :, :]
                pls = [ps.tile([128, DW], F32, tag="acc", bufs=4,
                               name=f"pl{jb}_{dh}") for dh in range(DH)]
                for kp in range(KT // 2):
                    for dh in range(DH):
                        # dh-inner: consecutive matmuls share the lhsT load
                        nc.tensor.matmul(
                            pls[dh],
                            lhsT=la[:, 2 * kp:2 * kp + 2, :],
                            rhs=hlp8_res[:, 2 * kp:2 * kp + 2,
                                         dh * DW:(dh + 1) * DW],
                            start=(kp == 0), stop=(kp == KT // 2 - 1),
                            perf_mode=DROW)
                if jb + 2 < PB:
                    load_la(jb + 2)
                for dh in range(DH):
                    st = out_st[:, (jb * DH + dh) % 3, :DW]
                    nc.scalar.activation(
                        out=st, in_=pls[dh], func=COPY, scale=-1.0 / S1)
                    nc.vector.tensor_add(
                        out=st, in0=st,
                        in1=hrn_b[:, jb, dh * DW:(dh + 1) * DW])
                    nc.sync.dma_start(
                        out=mu_lr_o[jb * 128:(jb + 1) * 128,
                                    dh * DW:(dh + 1) * DW], in_=st)

        # dummy writes for any output a stopped-early build didn't produce
        if LVL < 3:
            for pb in range(PB):
                for dd in range(DH):
                    st = out_st[:, pb % 3, :]
                    nc.vector.tensor_copy(
                        out=st, in_=hrn_b[:, pb, dd * DW:(dd + 1) * DW])
                    nc.sync.dma_start(
                        out=mu_lr_o[pb * 128:(pb + 1) * 128,
                                    dd * DW:(dd + 1) * DW], in_=st)
        if LVL < 2:
            for pb in range(PB):
                for dh in range(DH):
                    st = out_st[:, pb % 3, :]
                    nc.vector.tensor_copy(
                        out=st, in_=hl_nb[:, pb, dh * DW:(dh + 1) * DW])
                    nc.sync.dma_start(
                        out=mu_rl_o[pb * 128:(pb + 1) * 128,
                                    dh * DW:(dh + 1) * DW], in_=st)

    nc.compile()
    return nc


_NC_CACHE = {}


def _get_nc():
    if "nc" not in _NC_CACHE:
        _NC_CACHE["nc"] = build(C=8, NL=1024, M=8192, D=1024)
    return _NC_CACHE["nc"]


def kernel(hl, hr):
    """Full inputs in, full outputs out; distributes across 8 cores."""
    from concourse.bass_utils import run_bass_kernel_spmd

    C, NL = 8, 1024
    hl = np.ascontiguousarray(np.asarray(hl, dtype=np.float32))
    hr = np.ascontiguousarray(np.asarray(hr, dtype=np.float32))
    nc = _get_nc()
    in_maps = [
        {"hl": np.ascontiguousarray(hl[c * NL:(c + 1) * NL]),
         "hr": np.ascontiguousarray(hr[c * NL:(c + 1) * NL])}
        for c in range(C)
    ]
    res = run_bass_kernel_spmd(nc, in_maps, list(range(C)))
    mu_lr = np.concatenate([res.results[c]["mu_lr"] for c in range(C)])
    mu_rl = np.concatenate([res.results[c]["mu_rl"] for c in range(C)])
    return mu_lr, mu_rl


# revision 34
# speedup vs baseline: 1.0622x; 1.0622x over previous
"""Self-contained TRN2 kernel for the bidirectional attention correction.

kernel(hl, hr) -> (mu_lr, mu_rl), matching:
    hl_n = rownorm(hl); hr_n = rownorm(hr)
    a = hl_n @ hr_n.T
    mu_lr = hr_n - softmax(a, 1).T @ hl_n
    mu_rl = hl_n - softmax(a, 0) @ hr_n

SPMD on 8 NeuronCores: core c owns rows [c*1024,(c+1)*1024) of hl and hr.

Structure (all matmuls fp8 DoubleRow, 157 TF/s):
  prep: bulk-load hl/hr, row-normalize, transpose; AllGather hr_n.T (fp8,
        2 halves, first fired early).
  P1:   per 512-col chunk: a-matmuls, exp into a small rotating buffer,
        column-sum matmuls, slab DMA to the AllToAll staging, and PE
        transposes into the resident exp_aT [j-part, i].  Natural-layout
        exp lives only per-chunk.  The h0-half collectives (ReduceScatter
        of col sums, AllToAll) fire mid-P1.
  P2b:  mu_rl = hl_n - (exp_aT contract @ hrn8s)/S2P -- pure DR matmuls
        (lhsT = exp_aT slices), rhs = 1/s-scaled hr_n fp8 gathered; h0
        contraction first while the h1 collectives land.
  P2a:  mu_lr = hr_n - (a2a_exp contract @ hlp8)/S1 -- lhsT streamed from
        the AllToAll output, rhs = r-scaled hl_n fp8 gathered.
"""

import sys

for _p in ("/opt/trn_rl_repo",):
    if _p not in sys.path:
        sys.path.insert(0, _p)

from contextlib import ExitStack

import numpy as np

import concourse.bass as bass
import concourse.tile as tile
from concourse import bacc, mybir
from concourse.masks import make_identity

F32 = mybir.dt.float32
BF16 = mybir.dt.bfloat16
FP8 = mybir.dt.float8e4

ADD = mybir.AluOpType.add
SUB = mybir.AluOpType.subtract
MULT = mybir.AluOpType.mult
BYPASS = mybir.AluOpType.bypass
EXP = mybir.ActivationFunctionType.Exp
COPY = mybir.ActivationFunctionType.Copy
SQUARE = mybir.ActivationFunctionType.Square
AXL_X = mybir.AxisListType.X
DROW = mybir.MatmulPerfMode.DoubleRow


def build(C=8, NL=1024, M=8192, D=1024, stop_after="full"):
    PB = NL // 128          # local row blocks (i)
    DK = D // 128           # contraction chunks over D
    JB = M // 128           # j 128-blocks
    BLK = M // C            # j-cols per core block (== NL here)
    NLH = NL // 2           # j-cols per half
    W1 = 512                # P1 j-chunk width
    NQ = NLH // W1          # chunks per (half, block) piece
    JC = M // W1            # P1 j-chunks
    DW = 512                # d-chunk width for P2 outputs
    DH = D // DW            # d-halves
    S1 = float(8 * M)       # hl' fp8 scale (P2a rhs)
    S2P = float(8 * M)      # hrn8s fp8 scale (P2b rhs, folded 1/s_j)
    SL = 16.0               # hl_n.T fp8 scale (P1 lhsT)
    SR = 16.0               # hr_n.T fp8 scale (P1 rhs)
    KT = C * PB             # 128-blocks along the full i axis
    PBH = PB // 2
    TR_INLINE = 14          # chunks whose transposes run inside P1
    groups = [list(range(C))]
    LVL = {"prep": 0, "p1": 1, "p2b": 2, "full": 3}[stop_after]
    assert PB % 2 == 0 and NQ == 1

    nc = bacc.Bacc("TRN2", target_bir_lowering=False, debug=False, num_devices=C)

    hl_in = nc.dram_tensor("hl", [NL, D], F32, kind="ExternalInput").ap()
    hr_in = nc.dram_tensor("hr", [NL, D], F32, kind="ExternalInput").ap()
    mu_lr_o = nc.dram_tensor("mu_lr", [NL, D], F32, kind="ExternalOutput").ap()
    mu_rl_o = nc.dram_tensor("mu_rl", [NL, D], F32, kind="ExternalOutput").ap()

    with tile.TileContext(nc) as tc, ExitStack() as ctx:
        dram = ctx.enter_context(tc.tile_pool(name="dram", bufs=1, space="DRAM"))
        sb = ctx.enter_context(tc.tile_pool(name="sb", bufs=1))
        ps = ctx.enter_context(tc.tile_pool(name="ps", bufs=1, space="PSUM"))

        # ---- internal DRAM ----
        hrnT_loc = [dram.tile([D, NLH], FP8, name=f"hrnT_loc{h}")
                    for h in range(2)]
        hrnT_all = [dram.tile([C, D, NLH], FP8, name=f"hrnT_all{h}",
                              addr_space="Shared") for h in range(2)]
        a2a_in = [dram.tile([C, 128, PB, NLH], FP8, name=f"a2a_in{h}")
                  for h in range(2)]
        a2a_out = [dram.tile([C, 128, PB, NLH], FP8, name=f"a2a_out{h}")
                   for h in range(2)]
        s_loc = [dram.tile([C * NLH], F32, name=f"s_loc{h}") for h in range(2)]
        s_red = [dram.tile([NLH], F32, name=f"s_red{h}") for h in range(2)]
        hrn8s_loc = dram.tile([NL, D], FP8)
        hrn8s_all = [dram.tile([C, NLH, D], FP8, name=f"hrn8s_all{h}",
                               addr_space="Shared") for h in range(2)]
        hlp8_loc = dram.tile([NL, D], FP8)
        hlp8_all = dram.tile([C, NL, D], FP8, addr_space="Shared")
        warm = dram.tile([128], F32, name="warm")
        warm_o = dram.tile([C * 128], F32, name="warm_o",
                           addr_space="Shared")

        # ---- SBUF resident ----
        exp_aT = sb.tile([128, JB, NL], FP8, name="exp_aT")    # exp(a).T
        exp_rot = sb.tile([128, PB, 4, W1], FP8, name="exp_rot")
        hl_nb = sb.tile([128, PB, D], BF16, name="hl_nb")      # hl_n
        hrn_b = sb.tile([128, PB, D], BF16, name="hrn_b")      # hr_n local
        hl_nT = sb.tile([128, DK, NL], FP8, name="hl_nT")      # hl_n.T * SL
        bulk = sb.tile([128, PB // 2, D], F32, name="bulk")    # prep load
        rhsT = sb.tile([128, 2, DK, W1], FP8, name="rhsT")     # P1 rhs stream
        rhs_res = sb.tile([128, JB, DW], FP8, name="rhs_res",
                          tag="rhsT")                          # P2b rhs
        hlp8_res = sb.tile([128, KT, D], FP8, name="hlp8_res",
                           tag="exp_aT")                       # P2a rhs
        la_st = sb.tile([128, 2, KT, 128], FP8, name="la_st")  # P2a lhsT
        trT_st = sb.tile([128, 2, DK, 128], FP8, name="trT_st")
        cast_st = sb.tile([128, 4, D], FP8, name="cast_st")
        out_st = sb.tile([128, 3, DW], F32, name="out_st")
        s_row = sb.tile([1, 1, W1], F32, name="s_row")
        # consts / stats
        ident_b = sb.tile([128, 128], BF16, name="ident_b")
        ident_e = sb.tile([128, 128], FP8, name="ident_e")
        ones_e = sb.tile([128, 2, 16], FP8, name="ones_e")
        stats = sb.tile([128, 256], F32, name="stats")
        r_parts = stats[:, 0:PB * JC].rearrange("p (a b) -> p a b", a=PB)
        r_red = stats[:, 128:128 + PB]
        r_red3 = stats[:, 128:128 + PB].rearrange("p (a b) -> p a b", b=1)
        rinv = stats[:, 136:136 + PB]
        s_sb = stats[:, 144:144 + PB]
        srec = stats[:, 152:152 + PB]
        sinv = stats[:, 160:160 + PB]
        nrmp = stats[:, 168:168 + 2 * PB].rearrange(
            "p (a b) -> p a b", a=PB)                          # [128,PB,2]
        nrm1 = stats[:, 184:184 + PB]
        rnv_r = stats[:, 192:192 + PB]                         # 1/|hr row|
        rnv_l = stats[:, 200:200 + PB]                         # 1/|hl row|
        nrm2 = stats[:, 208:208 + PB]
        wsb = stats[:, 216:217]

        make_identity(nc, ident_b)
        nc.vector.tensor_copy(out=ident_e, in_=ident_b)
        nc.vector.memset(ones_e, 1.0)

        # tiny warm-up collective: absorbs the CC-path init cost while the
        # prep pipeline runs
        nc.sync.dma_start(
            out=warm.rearrange("(p a) -> p a", p=128), in_=wsb)
        nc.gpsimd.collective_compute(
            "AllGather", BYPASS, replica_groups=groups,
            ins=[warm.opt()], outs=[warm_o.opt()])

        # ================= prep: bulk norm + transpose ======================
        DQ = [nc.sync, nc.scalar]

        def norm_q(src, half, q, dstb, rnv):
            """Load one 128-row block on 2 queues, normalize (bf16)."""
            pb = half * PBH + q
            for dd in range(DH):
                DQ[dd].dma_start(
                    out=bulk[:, q, dd * DW:(dd + 1) * DW],
                    in_=src[pb * 128:(pb + 1) * 128, dd * DW:(dd + 1) * DW])
                sq = ps.tile([128, DW], F32, tag="acc", bufs=4,
                             name=f"sq{half}_{q}_{dd}")
                nc.scalar.activation(
                    out=sq, in_=bulk[:, q, dd * DW:(dd + 1) * DW],
                    func=SQUARE, accum_out=nrmp[:, pb, dd:dd + 1])
            # per-block latency chain: add, sqrt, recip on [128,1]
            nc.vector.tensor_add(
                out=nrm1[:, pb:pb + 1], in0=nrmp[:, pb, 0:1],
                in1=nrmp[:, pb, 1:2])
            nc.scalar.sqrt(out=nrm2[:, pb:pb + 1], in_=nrm1[:, pb:pb + 1])
            nc.vector.reciprocal(out=rnv[:, pb:pb + 1], in_=nrm2[:, pb:pb + 1])
            nc.vector.tensor_scalar_mul(
                out=dstb[:, pb, :], in0=bulk[:, q, :],
                scalar1=rnv[:, pb:pb + 1])

        def hr_transpose(pb):
            tstage = trT_st[:, pb % 2, :, :]
            for dk in range(DK):
                pst = ps.tile([128, 128], BF16, tag="rot", bufs=3,
                              name=f"ptB{pb}_{dk}")
                nc.tensor.transpose(
                    pst, hrn_b[:, pb, dk * 128:(dk + 1) * 128], ident_b)
                if dk % 2 == 0:
                    nc.scalar.mul(out=tstage[:, dk, :], in_=pst, mul=SR)
                else:
                    nc.vector.tensor_scalar_mul(
                        out=tstage[:, dk, :], in0=pst, scalar1=SR)
            h, pq = divmod(pb, PBH)
            nc.sync.dma_start(
                out=hrnT_loc[h].rearrange("(dk p) j -> p dk j", p=128)
                [:, :, pq * 128:(pq + 1) * 128],
                in_=tstage)

        def hl_transpose(pb):
            for dk in range(DK):
                pst = ps.tile([128, 128], BF16, tag="rot", bufs=3,
                              name=f"ptA{pb}_{dk}")
                nc.tensor.transpose(
                    pst, hl_nb[:, pb, dk * 128:(dk + 1) * 128], ident_b)
                nc.vector.tensor_scalar_mul(
                    out=hl_nT[:, dk, pb * 128:(pb + 1) * 128], in0=pst,
                    scalar1=SL)

        # hr half0 -> gather0 early; then hl half0, hr half1 -> gather1, hl h1
        for q in range(PBH):
            norm_q(hr_in, 0, q, hrn_b, rnv_r)
            hr_transpose(q)
        nc.gpsimd.collective_compute(
            "AllGather", BYPASS, replica_groups=groups,
            ins=[hrnT_loc[0].opt()], outs=[hrnT_all[0].opt()])
        for q in range(PBH):
            norm_q(hl_in, 0, q, hl_nb, rnv_l)
            hl_transpose(q)
        for q in range(PBH):
            norm_q(hr_in, 1, q, hrn_b, rnv_r)
            hr_transpose(PBH + q)
        nc.gpsimd.collective_compute(
            "AllGather", BYPASS, replica_groups=groups,
            ins=[hrnT_loc[1].opt()], outs=[hrnT_all[1].opt()])
        for q in range(PBH):
            norm_q(hl_in, 1, q, hl_nb, rnv_l)
            hl_transpose(PBH + q)

        # ---- post-half-h helpers ------------------------------------------
        def s_rs(h):
            """ReduceScatter the h-half col sums (dispatch only)."""
            nc.gpsimd.collective_compute(
                "ReduceScatter", ADD, replica_groups=groups,
                ins=[s_loc[h].opt()], outs=[s_red[h].opt()])

        def sinv_chain(h, readback=None):
            """s_red[h] -> sinv columns (readback + DVE recip/mul)."""
            pbs = slice(h * PBH, (h + 1) * PBH)
            (readback or nc.scalar).dma_start(
                out=s_sb[:, pbs],
                in_=s_red[h].rearrange("(b p) -> p b", p=128))
            nc.vector.reciprocal(out=srec[:, pbs], in_=s_sb[:, pbs])
            nc.vector.tensor_scalar_mul(
                out=sinv[:, pbs], in0=srec[:, pbs], scalar1=S2P)

        def hrn8s_half(h):
            """Scale own hr_n rows by sinv -> fp8 -> AllGather."""
            hrn8s_rows = hrn8s_loc.rearrange("(pb p) d -> p pb d", p=128)
            for pq in range(PBH):
                pb = h * PBH + pq
                st8 = cast_st[:, 2 + pb % 2, :]
                nc.scalar.activation(
                    out=st8, in_=hrn_b[:, pb, :], func=COPY,
                    scale=sinv[:, pb:pb + 1])
                nc.gpsimd.dma_start(out=hrn8s_rows[:, pb, :], in_=st8)
            nc.gpsimd.collective_compute(
                "AllGather", BYPASS, replica_groups=groups,
                ins=[hrn8s_loc[h * NLH:(h + 1) * NLH, :].opt()],
                outs=[hrn8s_all[h].opt()])

        def load_rhs_half(dh, engine, halves=(0, 1)):
            for b in range(C):
                for h in halves:
                    j0b = b * PB + h * PBH
                    engine.dma_start(
                        out=rhs_res[:, j0b:j0b + PBH, :],
                        in_=hrn8s_all[h][b].rearrange(
                            "(jb p) d -> p jb d", p=128)
                        [:, :, dh * DW:(dh + 1) * DW])

        def a2a_half(h):
            nc.gpsimd.collective_compute(
                "AllToAll", BYPASS, replica_groups=groups,
                ins=[a2a_in[h].opt()], outs=[a2a_out[h].opt()])

        def rot_slot(ci):
            # deferred-transposed chunks keep their exp in dedicated slots
            return ci % 2 if ci < TR_INLINE else 2 + (ci - TR_INLINE)

        def transpose_chunk(ci):
            """Transpose chunk ci's exp block into exp_aT (PE + DVE/Act)."""
            h, b = divmod(ci, C)
            sl = rot_slot(ci)
            for ib in range(PB):
                for jq in range(W1 // 128):
                    jb = (b * BLK + h * NLH) // 128 + jq
                    pst = ps.tile([128, 128, 2], FP8, tag="rot", bufs=3,
                                  name=f"ptb{ci}_{ib}_{jq}")
                    nc.tensor.transpose(
                        pst[:, :, 0],
                        exp_rot[:, ib, sl, jq * 128:(jq + 1) * 128], ident_e)
                    if (ib + jq) % 4 < 3:
                        nc.vector.tensor_copy(
                            out=exp_aT[:, jb, ib * 128:(ib + 1) * 128],
                            in_=pst[:, :, 0])
                    else:
                        nc.scalar.copy(
                            out=exp_aT[:, jb, ib * 128:(ib + 1) * 128],
                            in_=pst[:, :, 0])

        # ================= P1: a-matmul + exp + col sums + transposes =======
        chunks = [(h, b, q) for h in range(2) for b in range(C)
                  for q in range(NQ)]
        if LVL >= 1:
            deferred = []

            def flush_deferred():
                while deferred:
                    deferred.pop(0)()

            psum_s = [None]

            for ci, (h, b, q) in enumerate(chunks):
                j0 = b * BLK + h * NLH + q * W1   # global j of this chunk
                sl = rot_slot(ci)
                rt = rhsT[:, ci % 2, :, :]
                nc.sync.dma_start(
                    out=rt,
                    in_=hrnT_all[h][b].rearrange("(dk p) j -> p dk j", p=128)
                    [:, :, q * W1:(q + 1) * W1])
                for ib in range(PB):
                    t_ = ci * PB + ib
                    pa = ps.tile([128, W1], F32,
                                 tag=("rot" if t_ % 7 < 3 else "acc"),
                                 bufs=(3 if t_ % 7 < 3 else 4),
                                 name=f"pa{ci}_{ib}")
                    for dkp in range(DK // 2):
                        nc.tensor.matmul(
                            pa,
                            lhsT=hl_nT[:, 2 * dkp:2 * dkp + 2,
                                       ib * 128:(ib + 1) * 128],
                            rhs=rt[:, 2 * dkp:2 * dkp + 2, :],
                            start=(dkp == 0), stop=(dkp == DK // 2 - 1),
                            perf_mode=DROW)
                    et = exp_rot[:, ib, sl, :]
                    nc.scalar.activation(
                        out=et, in_=pa, func=EXP, scale=1.0 / (SL * SR),
                        accum_out=r_parts[:, ib, ci:ci + 1])
                    if ib % 2 == 0:
                        flush_deferred()
                        continue

                    def ones_mm(ci=ci, ib=ib, h=h, b=b, sl=sl):
                        ibp = ib // 2
                        if ibp == 0:
                            psum_s[0] = ps.tile([1, W1], F32, tag="colsum",
                                                bufs=1, name=f"pscs{ci}")
                        # DoubleRow column-sum over an i-pair
                        nc.tensor.matmul(
                            psum_s[0], lhsT=ones_e[:, :, 0:1],
                            rhs=exp_rot[:, ib - 1:ib + 1, sl, :],
                            start=(ibp == 0), stop=(ibp == PB // 2 - 1),
                            perf_mode=DROW)
                        if ib == PB - 1:
                            sr = s_row[:, 0, :]
                            nc.vector.tensor_copy(out=sr, in_=psum_s[0])
                            nc.sync.dma_start(
                                out=s_loc[h][b * W1:(b + 1) * W1].rearrange(
                                    "(a b) -> a b", a=1),
                                in_=sr)

                    flush_deferred()
                    deferred.append(ones_mm)
                # exp chunk -> AllToAll staging slab
                nc.scalar.dma_start(
                    out=a2a_in[h][b],
                    in_=exp_rot[:, :, sl, :])
                flush_deferred()
                if ci < TR_INLINE:
                    transpose_chunk(ci)
                if ci == C - 1:
                    # h0 fully done: dispatch-safe h0 collectives during P1
                    s_rs(0)
                    a2a_half(0)

            # --- P1 end: per-engine-queue ordering matters here. ---
            s_rs(1)                      # gpsimd; CC after A2A-h0
            sinv_chain(0)                # scalar readback + DVE (deps met)
            hrn8s_half(0)                # Act casts h0 + AG-h0 dispatch
            load_rhs_half(0, engine=nc.sync, halves=(0,))
            for ci in range(TR_INLINE, JC):
                transpose_chunk(ci)      # fills the CC gap on PE
            # h1 chain after the transpose copies so the RS1 wait does not
            # head-of-line-block the Act/DVE queues
            sinv_chain(1, readback=nc.sync)
            hrn8s_half(1)                # Act casts h1 + AG-h1 dispatch
            load_rhs_half(0, engine=nc.sync, halves=(1,))

            # r -> rinv ; hl' fp8 -> DRAM -> gather (P2a rhs). DVE + sync.
            nc.vector.tensor_reduce(out=r_red3, in_=r_parts, op=ADD, axis=AXL_X)
            nc.vector.reciprocal(out=rinv, in_=r_red)
            hlp8_rows = hlp8_loc.rearrange("(pb p) d -> p pb d", p=128)
            for ib in range(PB):
                st8 = cast_st[:, ib % 2, :]
                nc.vector.tensor_scalar(
                    out=st8, in0=hl_nb[:, ib, :],
                    scalar1=rinv[:, ib:ib + 1], scalar2=S1, op0=MULT, op1=MULT)
                nc.sync.dma_start(out=hlp8_rows[:, ib, :], in_=st8)
            nc.gpsimd.collective_compute(
                "AllGather", BYPASS, replica_groups=groups,
                ins=[hlp8_loc.opt()], outs=[hlp8_all.opt()])
            a2a_half(1)

        # ====== P2b: mu_rl = hl_n - (exp_aT contract @ hrn8s)/S2P ===========
        # jbp order: h0 rows first so the h1 collectives can land meanwhile
        jbp_order = ([jbp for jbp in range(JB // 2) if jbp % PBH < 2]
                     + [jbp for jbp in range(JB // 2) if jbp % PBH >= 2])

        def load_la(jb):
            la = la_st[:, jb % 2, :, :]
            h, jo = divmod(jb, PBH)
            for src in range(C):
                nc.scalar.dma_start(
                    out=la[:, src * PB:(src + 1) * PB, :],
                    in_=a2a_out[h][src][:, :, jo * 128:(jo + 1) * 128])

        if LVL >= 2:
            for dh in range(DH):
                if dh > 0:
                    load_rhs_half(dh, engine=nc.sync)
                    if LVL >= 3:
                        # prefetch P2a streams while dh=1 computes: rhs
                        # (into the freeing exp_aT slot) + first 2 lhsT blocks
                        for src in range(C):
                            DQ[src % 2].dma_start(
                                out=hlp8_res[:, src * PB:(src + 1) * PB, :],
                                in_=hlp8_all[src].rearrange(
                                    "(ib p) d -> p ib d", p=128))
                        load_la(0)
                        load_la(1)
                for ib in range(PB):
                    acc = ps.tile([128, DW], F32, tag="acc", bufs=4,
                                  name=f"acc{dh}_{ib}")
                    for jx, jbp in enumerate(jbp_order):
                        # DoubleRow: contraction over a jb-pair (K=256)
                        nc.tensor.matmul(
                            acc,
                            lhsT=exp_aT[:, 2 * jbp:2 * jbp + 2,
                                        ib * 128:(ib + 1) * 128],
                            rhs=rhs_res[:, 2 * jbp:2 * jbp + 2, :],
                            start=(jx == 0), stop=(jx == JB // 2 - 1),
                            perf_mode=DROW)
                    st = out_st[:, (dh * PB + ib) % 3, :DW]
                    nc.scalar.activation(
                        out=st, in_=acc, func=COPY, scale=-1.0 / S2P)
                    nc.vector.tensor_add(
                        out=st, in0=st,
                        in1=hl_nb[:, ib, dh * DW:(dh + 1) * DW])
                    nc.gpsimd.dma_start(
                        out=mu_rl_o[ib * 128:(ib + 1) * 128,
                                    dh * DW:(dh + 1) * DW], in_=st)

        # ====== P2a: mu_lr = hr_n - (exp contract @ hlp8)/S1 ================
        if LVL >= 3:
            for jb in range(PB):
                la = la_st[:, jb % 2, :, :]
                pls = [ps.tile([128, DW], F32, tag="acc", bufs=4,
                               name=f"pl{jb}_{dh}") for dh in range(DH)]
                for kp in range(KT // 2):
                    for dh in range(DH):
                        # dh-inner: consecutive matmuls share the lhsT load
                        nc.tensor.matmul(
                            pls[dh],
                            lhsT=la[:, 2 * kp:2 * kp + 2, :],
                            rhs=hlp8_res[:, 2 * kp:2 * kp + 2,
                                         dh * DW:(dh + 1) * DW],
                            start=(kp == 0), stop=(kp == KT // 2 - 1),
                            perf_mode=DROW)
                if jb + 2 < PB:
                    load_la(jb + 2)
                for dh in range(DH):
                    st = out_st[:, (jb * DH + dh) % 3, :DW]
                    nc.scalar.activation(
                        out=st, in_=pls[dh], func=COPY, scale=-1.0 / S1)
                    nc.vector.tensor_add(
                        out=st, in0=st,
                        in1=hrn_b[:, jb, dh * DW:(dh + 1) * DW])
                    nc.sync.dma_start(
                        out=mu_lr_o[jb * 128:(jb + 1) * 128,
                                    dh * DW:(dh + 1) * DW], in_=st)

        # dummy writes for any output a stopped-early build didn't produce
        if LVL < 3:
            for pb in range(PB):
                for dd in range(DH):
                    st = out_st[:, pb % 3, :]
                    nc.vector.tensor_copy(
                        out=st, in_=hrn_b[:, pb, dd * DW:(dd + 1) * DW])
                    nc.sync.dma_start(
                        out=mu_lr_o[pb * 128:(pb + 1) * 128,
                                    dd * DW:(dd + 1) * DW], in_=st)
        if LVL < 2:
            for pb in range(PB):
                for dh in range(DH):
                    st = out_st[:, pb % 3, :]
                    nc.vector.tensor_copy(
                        out=st, in_=hl_nb[:, pb, dh * DW:(dh + 1) * DW])
                    nc.sync.dma_start(
                        out=mu_rl_o[pb * 128:(pb + 1) * 128,
                                    dh * DW:(dh + 1) * DW], in_=st)

    nc.compile()
    return nc


_NC_CACHE = {}


def _get_nc():
    if "nc" not in _NC_CACHE:
        _NC_CACHE["nc"] = build(C=8, NL=1024, M=8192, D=1024)
    return _NC_CACHE["nc"]


def kernel(hl, hr):
    """Full inputs in, full outputs out; distributes across 8 cores."""
    from concourse.bass_utils import run_bass_kernel_spmd

    C, NL = 8, 1024
    hl = np.ascontiguousarray(np.asarray(hl, dtype=np.float32))
    hr = np.ascontiguousarray(np.asarray(hr, dtype=np.float32))
    nc = _get_nc()
    in_maps = [
        {"hl": np.ascontiguousarray(hl[c * NL:(c + 1) * NL]),
         "hr": np.ascontiguousarray(hr[c * NL:(c + 1) * NL])}
        for c in range(C)
    ]
    res = run_bass_kernel_spmd(nc, in_maps, list(range(C)))
    mu_lr = np.concatenate([res.results[c]["mu_lr"] for c in range(C)])
    mu_rl = np.concatenate([res.results[c]["mu_rl"] for c in range(C)])
    return mu_lr, mu_rl
